# revision 13
# baseline (speedup 1.0000x reference)
"""Trainium2 Bass kernel for nn_Block_79164837200171 (dense transformer
block returning (x_out, attention_weights)).

Sharding: 8 cores = 4 batches x 2 row-halves (512 q-rows each), no
cross-core collectives.  For row-half j=1 the host rolls the token axis
by 512 so the device program is SPMD-uniform; the attention-weight
shard's k-axis is rolled back on the host.

Matmuls run in bf16 (weights pre-cast on host).  LayerNorm affine
params are folded into the following weight matrices on the host; all
biases enter PSUM via K=1 matmuls.  Attention processes HEAD PAIRS so
the two K=64 score matmuls occupy disjoint PE row-groups and run
concurrently.  o = w @ v uses the stationary-v form (out [64 d, 512 q],
N=512 keeps PE duty high); the softmax 1/Z lands on o via a K=1
broadcast matmul of the transposed reciprocal row.
"""

import numpy as np

import concourse.bass as bass
import concourse.mybir as mybir
from concourse.tile import TileContext
from concourse.bass_utils import run_bass_kernel_spmd

F32 = mybir.dt.float32
BF16 = mybir.dt.bfloat16

B, N, C, H, D = 4, 1024, 768, 12, 64
HID = 3072
R = 512              # q rows per core
NT = N // 128        # token tiles (8)
QT = R // 128        # q tiles per core (4)
CT = C // 128        # channel tiles (6)
KT = N // 128        # k tiles (8)
HT = HID // 128      # hidden tiles (24)
SCALE = D ** -0.5
LN_EPS = 1e-5


# --------------------------------------------------------------------------
# Workaround for this walrus snapshot: instructions hold at most ONE sync
# wait, but Tile's wait-assignment can stuff 2+ onto one instruction.
_counter = [0]


def _fixup_excess_waits(nc):
    n_fixed = 0
    for f in nc.m.functions:
        for bb in f.blocks:
            insts = list(bb.instructions)
            out = []
            changed = False
            for inst in insts:
                si = inst.sync_info
                waits = list(si.on_wait) if si and si.on_wait else []
                if len(waits) > 1:
                    changed = True
                    n_fixed += 1
                    si.on_wait = waits[:1]
                    for w in waits[1:]:
                        ev = mybir.InstEventSemaphore(
                            name=f"I-waitfix-{_counter[0]}",
                            engine=inst.engine,
                            ins=[],
                            outs=[],
                            sync_info=mybir.SyncInfo(on_wait=[w], on_update=[]),
                        )
                        _counter[0] += 1
                        out.append(ev)
                out.append(inst)
            if changed:
                bb.instructions = out
    return n_fixed


# --------------------------------------------------------------------------
def _build():
    nc = bass.Bass()
    P = {}
    P["x"] = nc.declare_dram_parameter("x", [N, C], F32, isOutput=False)
    P["w_qkv"] = nc.declare_dram_parameter("w_qkv", [C, 3 * C], BF16, isOutput=False)
    P["w_proj"] = nc.declare_dram_parameter("w_proj", [C, C], BF16, isOutput=False)
    P["w_fc1"] = nc.declare_dram_parameter("w_fc1", [C, HID], BF16, isOutput=False)
    P["w_fc2"] = nc.declare_dram_parameter("w_fc2", [HID, C], BF16, isOutput=False)
    P["b_qkv"] = nc.declare_dram_parameter("b_qkv", [1, 3 * C], BF16, isOutput=False)
    P["b_proj"] = nc.declare_dram_parameter("b_proj", [1, C], BF16, isOutput=False)
    P["b_fc1"] = nc.declare_dram_parameter("b_fc1", [1, HID], BF16, isOutput=False)
    P["b_fc2"] = nc.declare_dram_parameter("b_fc2", [1, C], BF16, isOutput=False)
    P["ident"] = nc.declare_dram_parameter("ident", [128, 128], BF16, isOutput=False)
    P["ident_f"] = nc.declare_dram_parameter("ident_f", [128, 128], F32, isOutput=False)
    P["ones"] = nc.declare_dram_parameter("ones", [1, 512], BF16, isOutput=False)
    P["mask_mul"] = nc.declare_dram_parameter("mask_mul", [3, 3], BF16, isOutput=False)
    P["mask_add"] = nc.declare_dram_parameter("mask_add", [3, 3], BF16, isOutput=False)
    wout = nc.declare_dram_parameter("wout", [H, R, N], F32, isOutput=True)
    xout = nc.declare_dram_parameter("xout", [R, C], F32, isOutput=True)

    Exp = mybir.ActivationFunctionType.Exp
    Gelu = mybir.ActivationFunctionType.Gelu
    Sqrt = mybir.ActivationFunctionType.Sqrt
    sub_ = mybir.AluOpType.subtract
    mul_ = mybir.AluOpType.mult

    with TileContext(nc) as tc:
        with tc.tile_pool(name="const", bufs=1) as pc, \
             tc.tile_pool(name="pers", bufs=1) as pp, \
             tc.tile_pool(name="xp", bufs=1) as px:
            x_sb = [px.tile([128, C], F32, tag=f"x{t}", name=f"x{t}")
                    for t in range(QT)]
            for t in range(QT):
                nc.sync.dma_start(out=x_sb[t],
                                  in_=P["x"][t * 128:(t + 1) * 128, :])
            ident = pc.tile([128, 128], BF16, tag="ident")
            ident_f = pc.tile([128, 128], F32, tag="ident_f")
            ones = pc.tile([1, 512], BF16, tag="ones")
            mask_mul = pc.tile([3, 3], BF16, tag="mask_mul")
            mask_add = pc.tile([3, 3], BF16, tag="mask_add")
            eps = pc.tile([128, 1], F32, tag="eps")
            b_qkv = pc.tile([1, 3 * C], BF16, tag="b_qkv")
            b_proj = pc.tile([1, C], BF16, tag="b_proj")
            b_fc1 = pc.tile([1, HID], BF16, tag="b_fc1")
            b_fc2 = pc.tile([1, C], BF16, tag="b_fc2")
            nc.sync.dma_start(out=ident, in_=P["ident"][:, :])
            nc.sync.dma_start(out=ident_f, in_=P["ident_f"][:, :])
            nc.sync.dma_start(out=ones, in_=P["ones"][:, :])
            nc.sync.dma_start(out=mask_mul, in_=P["mask_mul"][:, :])
            nc.sync.dma_start(out=mask_add, in_=P["mask_add"][:, :])
            nc.sync.dma_start(out=b_qkv, in_=P["b_qkv"][:, :])
            nc.sync.dma_start(out=b_proj, in_=P["b_proj"][:, :])
            nc.sync.dma_start(out=b_fc1, in_=P["b_fc1"][:, :])
            nc.sync.dma_start(out=b_fc2, in_=P["b_fc2"][:, :])
            nc.vector.memset(eps, LN_EPS)

            x2 = [pp.tile([128, C], F32, tag=f"x2_{q}", name=f"x2_{q}")
                  for q in range(QT)]
            h2T = [pp.tile([128, R], BF16, tag=f"h2T_{c}", name=f"h2T_{c}")
                   for c in range(CT)]

            def layernorm_to(xt, xh_pool, ps_tr):
                stats = xh_pool.tile([128, 3, 6], F32, tag="st", name="st")
                x3 = xt.rearrange("p (s c) -> p s c", c=256)
                for s in range(3):
                    nc.vector.bn_stats(out=stats[:, s, :], in_=x3[:, s, :])
                mv = xh_pool.tile([128, 2], F32, tag="mv", name="mv")
                nc.vector.bn_aggr(out=mv, in_=stats)
                rstd = xh_pool.tile([128, 1], F32, tag="rstd", name="rstd")
                nc.scalar.activation(rstd, mv[:, 1:2], Sqrt, bias=eps)
                nc.vector.reciprocal(rstd, rstd)
                nbias = xh_pool.tile([128, 1], F32, tag="nb", name="nb")
                nc.vector.tensor_scalar(out=nbias, in0=mv[:, 0:1],
                                        scalar1=rstd, scalar2=-1.0,
                                        op0=mul_, op1=mul_)
                xh = xh_pool.tile([128, C], BF16, tag="xh", name="xh")
                nc.scalar.activation(xh, xt,
                                     mybir.ActivationFunctionType.Identity,
                                     bias=nbias, scale=rstd)
                res = []
                for ci in range(CT):
                    pt = ps_tr.tile([128, 128], BF16, tag="tr", name="tr")
                    nc.tensor.transpose(pt, xh[:, ci * 128:(ci + 1) * 128], ident)
                    res.append(pt)
                return res

            # ---------------- phase 1+2: LN1 -> hT; QKV -> kT, v, qT -----
            with tc.tile_pool(name="kvq", bufs=1) as pk:
                kT = [pk.tile([128, N], BF16, tag=f"kT{c}", name=f"kT{c}")
                      for c in range(CT)]
                vv = [pk.tile([128, C], BF16, tag=f"v{t}", name=f"v{t}")
                      for t in range(NT)]
                qT = [pk.tile([128, R], BF16, tag=f"qT{c}", name=f"qT{c}")
                      for c in range(CT)]
                oT = [pk.tile([128, R], BF16, tag=f"oT{c}", name=f"oT{c}")
                      for c in range(CT)]
                qTz = [[pk.tile([128, R], BF16, tag=f"qTz{c}_{h}",
                                name=f"qTz{c}_{h}") for h in range(2)]
                       for c in range(CT)]
                for c in range(CT):
                    for h in range(2):
                        nc.vector.memset(qTz[c][h][(1 - h) * 64:(1 - h) * 64 + 64, :], 0.0)
                with tc.tile_pool(name="wqkv", bufs=1) as pwq, \
                     tc.tile_pool(name="hT", bufs=1) as ph:
                    wq_sb = [pwq.tile([128, 3 * C], BF16, tag=f"wq{c}",
                                      name=f"wq{c}") for c in range(CT)]
                    for ci in range(CT):
                        nc.sync.dma_start(
                            out=wq_sb[ci],
                            in_=P["w_qkv"][ci * 128:(ci + 1) * 128, :])
                    hT = [ph.tile([128, N], BF16, tag=f"hT{c}",
                                  name=f"hT{c}") for c in range(CT)]
                    with tc.tile_pool(name="ln1", bufs=2) as pl, \
                         tc.tile_pool(name="ln1ps", bufs=2, space="PSUM") as pltr:
                        for t in range(NT):
                            if t < QT:
                                xt = x_sb[t]
                            else:
                                xt = pl.tile([128, C], F32, tag="xkv",
                                             name="xkv")
                                nc.sync.dma_start(
                                    out=xt, in_=P["x"][t * 128:(t + 1) * 128, :])
                            pts = layernorm_to(xt, pl, pltr)
                            for ci in range(CT):
                                nc.scalar.copy(
                                    hT[ci][:, t * 128:(t + 1) * 128], pts[ci])

                    with tc.tile_pool(name="qkps", bufs=2, space="PSUM") as pq:
                        # v first: out [tok 128, vcol] (attention o-mms need it)
                        for t in range(NT):
                            for half in range(2):
                                ps = pq.tile([128, 384], F32, tag="pv",
                                             name="pv", bufs=3)
                                nc.tensor.matmul(
                                    ps, ones[0:1, 0:128],
                                    b_qkv[0:1, 2 * C + half * 384:
                                          2 * C + (half + 1) * 384],
                                    start=True, stop=False)
                                for ci in range(CT):
                                    nc.tensor.matmul(
                                        ps, hT[ci][:, t * 128:(t + 1) * 128],
                                        wq_sb[ci][:, 2 * C + half * 384:
                                                  2 * C + (half + 1) * 384],
                                        start=False, stop=(ci == CT - 1))
                                nc.vector.tensor_copy(
                                    vv[t][:, half * 384:(half + 1) * 384], ps)
                        # per ct: kT halves + kTz variants, then qT + qTz
                        for kc in range(CT):
                            for half in range(2):
                                ps = pq.tile([128, 512], F32, tag="pk",
                                             name="pk", bufs=3)
                                nc.tensor.matmul(
                                    ps,
                                    b_qkv[0:1, C + kc * 128:C + (kc + 1) * 128],
                                    ones, start=True, stop=False)
                                for ci in range(CT):
                                    nc.tensor.matmul(
                                        ps,
                                        wq_sb[ci][:, C + kc * 128:C + (kc + 1) * 128],
                                        hT[ci][:, half * 512:(half + 1) * 512],
                                        start=False, stop=(ci == CT - 1))
                                nc.vector.tensor_copy(
                                    kT[kc][:, half * 512:(half + 1) * 512], ps)
                            ps = pq.tile([128, 512], F32, tag="pq", name="pq")
                            nc.tensor.matmul(
                                ps, b_qkv[0:1, kc * 128:(kc + 1) * 128],
                                ones, start=True, stop=False)
                            for ci in range(CT):
                                nc.tensor.matmul(
                                    ps, wq_sb[ci][:, kc * 128:(kc + 1) * 128],
                                    hT[ci][:, 0:512],
                                    start=False, stop=(ci == CT - 1))
                            nc.scalar.activation(
                                qT[kc], ps, mybir.ActivationFunctionType.Copy,
                                scale=float(SCALE))
                            for hi in range(2):
                                nc.scalar.activation(
                                    qTz[kc][hi][hi * 64:hi * 64 + 64, :],
                                    ps[hi * 64:hi * 64 + 64, :],
                                    mybir.ActivationFunctionType.Copy,
                                    scale=float(SCALE))

                # proj/fc weight DMAs are interleaved between attention
                # head-pairs below so they don't block the wout queue
                with tc.tile_pool(name="wlate", bufs=1) as pwl:
                    wp_sb = [pwl.tile([128, C], BF16, tag=f"wp{c}",
                                      name=f"wp{c}") for c in range(CT)]
                    wf1_sb = [pwl.tile([128, HID], BF16, tag=f"wf1_{c}",
                                       name=f"wf1_{c}") for c in range(CT)]
                    wf2_sb = [pwl.tile([128, C], BF16, tag=f"wf2_{t}",
                                       name=f"wf2_{t}") for t in range(HT)]
                    wjobs = (
                        [(wp_sb[c], P["w_proj"][c * 128:(c + 1) * 128, :])
                         for c in range(CT)]
                        + [(wf1_sb[c], P["w_fc1"][c * 128:(c + 1) * 128, :])
                           for c in range(CT)]
                        + [(wf2_sb[t], P["w_fc2"][t * 128:(t + 1) * 128, :])
                           for t in range(HT)]
                    )

                    # ---------------- phase 3: attention (head pairs) -----
                    with tc.tile_pool(name="att", bufs=4) as pa, \
                         tc.tile_pool(name="attE", bufs=4) as pe, \
                         tc.tile_pool(name="aps", bufs=1, space="PSUM") as pps, \
                         tc.tile_pool(name="apo", bufs=1, space="PSUM") as ppo:
                        nw = (len(wjobs) + H // 2 - 1) // (H // 2)
                        for hp in range(H // 2):
                            ct = hp
                            heads = (2 * hp, 2 * hp + 1)
                            for dst, src in wjobs[hp * nw:(hp + 1) * nw]:
                                nc.sync.dma_start(out=dst, in_=src)
                            # ---- q path: scores -> exp(+Z) -> norm -> DMA
                            zs = {}
                            for qt in range(QT):
                                pq2 = {}
                                for hi in range(2):
                                    pq2[hi] = pps.tile([128, N], F32,
                                                       tag=f"sq{hi}",
                                                       name=f"sq{hi}", bufs=1)
                                for half in range(2):
                                    for hi in range(2):
                                        nc.tensor.matmul(
                                            pq2[hi][:, half * 512:(half + 1) * 512],
                                            qTz[ct][hi][:, qt * 128:(qt + 1) * 128],
                                            kT[ct][:, half * 512:(half + 1) * 512],
                                            start=True, stop=True)
                                for hi in range(2):
                                    if qt == 0:
                                        nc.vector.tensor_mul(
                                            pq2[hi][0:3, 0:3], pq2[hi][0:3, 0:3],
                                            mask_mul)
                                    e = pa.tile([128, N], F32, tag="e", name="e")
                                    z = pa.tile([128, 1], F32, tag="z",
                                                name="z", bufs=10)
                                    nc.scalar.activation(e, pq2[hi], Exp,
                                                         accum_out=z)
                                    nc.vector.reciprocal(z, z)
                                    zs[(qt, hi)] = z
                                    nc.vector.tensor_scalar_mul(e, in0=e,
                                                                scalar1=z)
                                    nc.sync.dma_start(
                                        out=wout[heads[hi],
                                                 qt * 128:(qt + 1) * 128, :],
                                        in_=e)
                            # ---- 1/Z broadcast rows [64, 512] per head
                            zbc = {}
                            for hi in range(2):
                                zrow = pa.tile([1, 512], BF16, tag=f"zr{hi}",
                                               name=f"zr{hi}", bufs=2)
                                for qt in range(QT):
                                    pzt = ppo.tile([1, 128], F32,
                                                   tag="zoo",
                                                   name=f"zt{hi}", bufs=2)
                                    nc.tensor.matmul(pzt, zs[(qt, hi)], ident_f,
                                                     is_transpose=True,
                                                     start=True, stop=True)
                                    nc.vector.tensor_copy(
                                        zrow[0:1, qt * 128:(qt + 1) * 128], pzt)
                                pbc = ppo.tile([64, 512], F32, tag="zoo",
                                               name=f"bc{hi}", bufs=2)
                                nc.tensor.matmul(pbc, ones[0:1, 0:64], zrow,
                                                 start=True, stop=True)
                                zb = pa.tile([64, 512], F32, tag=f"zbc{hi}",
                                             name=f"zbc{hi}", bufs=2)
                                nc.vector.tensor_copy(zb, pbc)
                                zbc[hi] = zb
                            # ---- T path + o accumulation, per k-tile
                            so_pair = ppo.tile([128, 512], F32, tag="zoo",
                                               name="so_pair", bufs=2)
                            so = {0: so_pair[0:64, :], 1: so_pair[64:128, :]}
                            for kt in range(KT):
                                pss = []
                                for hi in range(2):
                                    ps = pps.tile([128, 512], F32,
                                                  tag=f"sT{hi}", name=f"sT{hi}",
                                                  bufs=1)
                                    nc.tensor.matmul(
                                        ps,
                                        kT[ct][:, kt * 128:(kt + 1) * 128],
                                        qTz[ct][hi],
                                        start=True, stop=True)
                                    pss.append(ps)
                                ets = []
                                for hi in range(2):
                                    e_t = pe.tile([128, 512], BF16, tag="eT",
                                                  name="eT")
                                    nc.scalar.activation(e_t, pss[hi], Exp)
                                    if kt == 0:
                                        nc.vector.tensor_mul(
                                            e_t[0:3, 0:3], e_t[0:3, 0:3],
                                            mask_mul)
                                        nc.vector.tensor_add(
                                            e_t[0:3, 0:3], e_t[0:3, 0:3],
                                            mask_add)
                                    ets.append(e_t)
                                for hi in range(2):
                                    nc.tensor.matmul(
                                        so[hi],
                                        vv[kt][:, heads[hi] * 64:
                                               heads[hi] * 64 + 64],
                                        ets[hi],
                                        start=(kt == 0), stop=(kt == KT - 1),
                                        tile_position=(0, hi * 64))
                            # ---- normalize into oT
                            for hi in range(2):
                                nc.vector.tensor_mul(
                                    oT[ct][hi * 64:hi * 64 + 64, :],
                                    so[hi], zbc[hi])

                    # ------------- phase 4: proj + residual + LN2 ---------
                    with tc.tile_pool(name="prl", bufs=2) as pjl, \
                         tc.tile_pool(name="pjps", bufs=2, space="PSUM") as pjp:
                        for qt in range(QT):
                            for half, w0, w1 in ((0, 0, 512), (1, 512, 768)):
                                ps = pjp.tile([128, w1 - w0], F32,
                                              tag=f"pp{half}", name=f"pp{half}")
                                nc.tensor.matmul(ps, ones[0:1, 0:128],
                                                 b_proj[0:1, w0:w1],
                                                 start=True, stop=False)
                                for ci in range(CT):
                                    nc.tensor.matmul(
                                        ps, oT[ci][:, qt * 128:(qt + 1) * 128],
                                        wp_sb[ci][:, w0:w1],
                                        start=False, stop=(ci == CT - 1))
                                nc.vector.tensor_add(
                                    x2[qt][:, w0:w1], ps, x_sb[qt][:, w0:w1])
                            pts = layernorm_to(x2[qt], pjl, pjp)
                            for ci in range(CT):
                                nc.vector.tensor_copy(
                                    h2T[ci][:, qt * 128:(qt + 1) * 128],
                                    pts[ci])

                    # ---------------- phase 5: MLP ------------------------
                    with tc.tile_pool(name="mlg", bufs=1) as pg, \
                         tc.tile_pool(name="mps", bufs=2, space="PSUM") as pmp, \
                         tc.tile_pool(name="mpsa", bufs=1, space="PSUM") as pma:
                        gT = [pg.tile([128, R], BF16, tag=f"gT{t}",
                                      name=f"gT{t}") for t in range(HT)]
                        acc = [pma.tile([128, 384], F32, tag=f"acc{q}",
                                        name=f"acc{q}") for q in range(QT)]
                        for pass_i, w0, w1 in ((0, 0, 384), (1, 384, 768)):
                            for qt in range(QT):
                                nc.tensor.matmul(acc[qt], ones[0:1, 0:128],
                                                 b_fc2[0:1, w0:w1],
                                                 start=True, stop=False)
                            for ht in range(HT):
                                if pass_i == 0:
                                    ps = pmp.tile([128, 512], F32, tag="f1",
                                                  name="f1")
                                    nc.tensor.matmul(
                                        ps, b_fc1[0:1, ht * 128:(ht + 1) * 128],
                                        ones, start=True, stop=False)
                                    for ci in range(CT):
                                        nc.tensor.matmul(
                                            ps,
                                            wf1_sb[ci][:, ht * 128:(ht + 1) * 128],
                                            h2T[ci],
                                            start=False, stop=(ci == CT - 1))
                                    nc.scalar.activation(gT[ht], ps, Gelu)
                                for qt in range(QT):
                                    nc.tensor.matmul(
                                        acc[qt],
                                        gT[ht][:, qt * 128:(qt + 1) * 128],
                                        wf2_sb[ht][:, w0:w1],
                                        start=False, stop=(ht == HT - 1))
                            for qt in range(QT):
                                nc.vector.tensor_add(x2[qt][:, w0:w1],
                                                     acc[qt],
                                                     x2[qt][:, w0:w1])
                                nc.sync.dma_start(
                                    out=xout[qt * 128:(qt + 1) * 128, w0:w1],
                                    in_=x2[qt][:, w0:w1])

    _fixup_excess_waits(nc)
    return nc


_NC = None


def _get_nc():
    global _NC
    if _NC is None:
        _NC = _build()
    return _NC


def _prep_in_maps(inputs):
    import ml_dtypes
    bf = ml_dtypes.bfloat16
    x = np.ascontiguousarray(np.asarray(inputs["x"], dtype=np.float32))
    ln1_g = np.asarray(inputs["ln1_g"], np.float32)
    ln1_b = np.asarray(inputs["ln1_b"], np.float32)
    ln2_g = np.asarray(inputs["ln2_g"], np.float32)
    ln2_b = np.asarray(inputs["ln2_b"], np.float32)
    w_qkv = np.asarray(inputs["w_qkv"], np.float32)
    w_proj = np.asarray(inputs["w_proj"], np.float32)
    b_proj = np.asarray(inputs["b_proj"], np.float32)
    w_fc1 = np.asarray(inputs["w_fc1"], np.float32)
    b_fc1 = np.asarray(inputs["b_fc1"], np.float32)
    w_fc2 = np.asarray(inputs["w_fc2"], np.float32)
    b_fc2 = np.asarray(inputs["b_fc2"], np.float32)

    # fold LN affine into following matmuls
    w_qkv_f = w_qkv * ln1_g[:, None]
    b_qkv_f = (ln1_b @ w_qkv)[None, :]
    w_fc1_f = w_fc1 * ln2_g[:, None]
    b_fc1_f = (b_fc1 + ln2_b @ w_fc1)[None, :]

    cc = np.ascontiguousarray
    eye3 = np.eye(3, dtype=np.float32)
    common = dict(
        w_qkv=cc(w_qkv_f.astype(bf)), w_proj=cc(w_proj.astype(bf)),
        w_fc1=cc(w_fc1_f.astype(bf)), w_fc2=cc(w_fc2.astype(bf)),
        b_qkv=cc(b_qkv_f.astype(bf)), b_proj=cc(b_proj[None, :].astype(bf)),
        b_fc1=cc(b_fc1_f.astype(bf)), b_fc2=cc(b_fc2[None, :].astype(bf)),
        ident=np.eye(128, dtype=np.float32).astype(bf),
        ident_f=np.eye(128, dtype=np.float32),
        ones=np.ones((1, 512), np.float32).astype(bf),
    )
    in_maps = []
    for core in range(8):
        b, j = core // 2, core % 2
        xb = x[b]
        if j == 1:
            xb = np.ascontiguousarray(np.concatenate([xb[R:], xb[:R]], axis=0))
        m = dict(common)
        m["x"] = np.ascontiguousarray(xb)
        if j == 0:
            m["mask_mul"] = cc(eye3.astype(bf))
            m["mask_add"] = cc((1.0 - eye3).astype(bf))
        else:
            m["mask_mul"] = cc(np.ones((3, 3), np.float32).astype(bf))
            m["mask_add"] = cc(np.zeros((3, 3), np.float32).astype(bf))
        in_maps.append(m)
    return in_maps


def _assemble(results):
    x_out = np.empty((B, N, C), np.float32)
    weights = np.empty((B, H, N, N), np.float32)
    for core in range(8):
        b, j = core // 2, core % 2
        r = results[core]
        x_out[b, j * R:(j + 1) * R] = r["xout"]
        w = r["wout"]
        if j == 1:
            w = np.concatenate([w[:, :, R:], w[:, :, :R]], axis=2)
        weights[b, :, j * R:(j + 1) * R, :] = w
    return x_out, weights


def run(inputs, trace=False, tmpdir=None):
    nc = _get_nc()
    in_maps = _prep_in_maps(inputs)
    res = run_bass_kernel_spmd(nc, in_maps, core_ids=list(range(8)),
                               trace=trace, tmpdir=tmpdir)
    return _assemble(res.results), res


def kernel(**inputs):
    (x_out, weights), _ = run(inputs)
    return x_out, weights


# revision 14
# speedup vs baseline: 1.0088x; 1.0088x over previous
"""Trainium2 Bass kernel for nn_Block_79164837200171 (dense transformer
block returning (x_out, attention_weights)).

Sharding: 8 cores = 4 batches x 2 row-halves (512 q-rows each), no
cross-core collectives.  For row-half j=1 the host rolls the token axis
by 512 so the device program is SPMD-uniform; the attention-weight
shard's k-axis is rolled back on the host.

Matmuls run in bf16 (weights pre-cast on host).  LayerNorm affine
params are folded into the following weight matrices on the host; all
biases enter PSUM via K=1 matmuls.  Attention processes HEAD PAIRS so
the two K=64 score matmuls occupy disjoint PE row-groups and run
concurrently.  o = w @ v uses the stationary-v form (out [64 d, 512 q],
N=512 keeps PE duty high); the softmax 1/Z lands on o via a K=1
broadcast matmul of the transposed reciprocal row.
"""

import numpy as np

import concourse.bass as bass
import concourse.mybir as mybir
from concourse.tile import TileContext
from concourse.bass_utils import run_bass_kernel_spmd

F32 = mybir.dt.float32
BF16 = mybir.dt.bfloat16

B, N, C, H, D = 4, 1024, 768, 12, 64
HID = 3072
R = 512              # q rows per core
NT = N // 128        # token tiles (8)
QT = R // 128        # q tiles per core (4)
CT = C // 128        # channel tiles (6)
KT = N // 128        # k tiles (8)
HT = HID // 128      # hidden tiles (24)
SCALE = D ** -0.5
LN_EPS = 1e-5


# --------------------------------------------------------------------------
# Workaround for this walrus snapshot: instructions hold at most ONE sync
# wait, but Tile's wait-assignment can stuff 2+ onto one instruction.
_counter = [0]


def _fixup_excess_waits(nc):
    n_fixed = 0
    for f in nc.m.functions:
        for bb in f.blocks:
            insts = list(bb.instructions)
            out = []
            changed = False
            for inst in insts:
                si = inst.sync_info
                waits = list(si.on_wait) if si and si.on_wait else []
                if len(waits) > 1:
                    changed = True
                    n_fixed += 1
                    si.on_wait = waits[:1]
                    for w in waits[1:]:
                        ev = mybir.InstEventSemaphore(
                            name=f"I-waitfix-{_counter[0]}",
                            engine=inst.engine,
                            ins=[],
                            outs=[],
                            sync_info=mybir.SyncInfo(on_wait=[w], on_update=[]),
                        )
                        _counter[0] += 1
                        out.append(ev)
                out.append(inst)
            if changed:
                bb.instructions = out
    return n_fixed


# --------------------------------------------------------------------------
def _build():
    nc = bass.Bass()
    P = {}
    P["x"] = nc.declare_dram_parameter("x", [N, C], F32, isOutput=False)
    P["w_qkv"] = nc.declare_dram_parameter("w_qkv", [C, 3 * C], BF16, isOutput=False)
    P["w_proj"] = nc.declare_dram_parameter("w_proj", [C, C], BF16, isOutput=False)
    P["w_fc1"] = nc.declare_dram_parameter("w_fc1", [C, HID], BF16, isOutput=False)
    P["w_fc2"] = nc.declare_dram_parameter("w_fc2", [HID, C], BF16, isOutput=False)
    P["b_qkv"] = nc.declare_dram_parameter("b_qkv", [1, 3 * C], BF16, isOutput=False)
    P["b_proj"] = nc.declare_dram_parameter("b_proj", [1, C], BF16, isOutput=False)
    P["b_fc1"] = nc.declare_dram_parameter("b_fc1", [1, HID], BF16, isOutput=False)
    P["b_fc2"] = nc.declare_dram_parameter("b_fc2", [1, C], BF16, isOutput=False)
    P["ident"] = nc.declare_dram_parameter("ident", [128, 128], BF16, isOutput=False)
    P["ident_f"] = nc.declare_dram_parameter("ident_f", [128, 128], F32, isOutput=False)
    P["ones"] = nc.declare_dram_parameter("ones", [1, 512], BF16, isOutput=False)
    P["mask_mul"] = nc.declare_dram_parameter("mask_mul", [3, 3], BF16, isOutput=False)
    P["mask_add"] = nc.declare_dram_parameter("mask_add", [3, 3], BF16, isOutput=False)
    wout = nc.declare_dram_parameter("wout", [H, R, N], F32, isOutput=True)
    xout = nc.declare_dram_parameter("xout", [R, C], F32, isOutput=True)

    Exp = mybir.ActivationFunctionType.Exp
    Gelu = mybir.ActivationFunctionType.Gelu
    Sqrt = mybir.ActivationFunctionType.Sqrt
    sub_ = mybir.AluOpType.subtract
    mul_ = mybir.AluOpType.mult

    with TileContext(nc) as tc:
        with tc.tile_pool(name="const", bufs=1) as pc, \
             tc.tile_pool(name="pers", bufs=1) as pp, \
             tc.tile_pool(name="xp", bufs=1) as px:
            x_sb = [px.tile([128, C], F32, tag=f"x{t}", name=f"x{t}")
                    for t in range(QT)]
            for t in range(QT):
                nc.sync.dma_start(out=x_sb[t],
                                  in_=P["x"][t * 128:(t + 1) * 128, :])
            ident = pc.tile([128, 128], BF16, tag="ident")
            ident_f = pc.tile([128, 128], F32, tag="ident_f")
            ones = pc.tile([1, 512], BF16, tag="ones")
            mask_mul = pc.tile([3, 3], BF16, tag="mask_mul")
            mask_add = pc.tile([3, 3], BF16, tag="mask_add")
            eps = pc.tile([128, 1], F32, tag="eps")
            b_qkv = pc.tile([1, 3 * C], BF16, tag="b_qkv")
            b_proj = pc.tile([1, C], BF16, tag="b_proj")
            b_fc1 = pc.tile([1, HID], BF16, tag="b_fc1")
            b_fc2 = pc.tile([1, C], BF16, tag="b_fc2")
            nc.sync.dma_start(out=ident, in_=P["ident"][:, :])
            nc.sync.dma_start(out=ident_f, in_=P["ident_f"][:, :])
            nc.sync.dma_start(out=ones, in_=P["ones"][:, :])
            nc.sync.dma_start(out=mask_mul, in_=P["mask_mul"][:, :])
            nc.sync.dma_start(out=mask_add, in_=P["mask_add"][:, :])
            nc.sync.dma_start(out=b_qkv, in_=P["b_qkv"][:, :])
            nc.sync.dma_start(out=b_proj, in_=P["b_proj"][:, :])
            nc.sync.dma_start(out=b_fc1, in_=P["b_fc1"][:, :])
            nc.sync.dma_start(out=b_fc2, in_=P["b_fc2"][:, :])
            nc.vector.memset(eps, LN_EPS)

            x2 = [pp.tile([128, C], F32, tag=f"x2_{q}", name=f"x2_{q}")
                  for q in range(QT)]
            h2T = [pp.tile([128, R], BF16, tag=f"h2T_{c}", name=f"h2T_{c}")
                   for c in range(CT)]

            def layernorm_to(xt, xh_pool, ps_tr):
                stats = xh_pool.tile([128, 3, 6], F32, tag="st", name="st")
                x3 = xt.rearrange("p (s c) -> p s c", c=256)
                for s in range(3):
                    nc.vector.bn_stats(out=stats[:, s, :], in_=x3[:, s, :])
                mv = xh_pool.tile([128, 2], F32, tag="mv", name="mv")
                nc.vector.bn_aggr(out=mv, in_=stats)
                rstd = xh_pool.tile([128, 1], F32, tag="rstd", name="rstd")
                nc.scalar.activation(rstd, mv[:, 1:2], Sqrt, bias=eps)
                nc.vector.reciprocal(rstd, rstd)
                nbias = xh_pool.tile([128, 1], F32, tag="nb", name="nb")
                nc.vector.tensor_scalar(out=nbias, in0=mv[:, 0:1],
                                        scalar1=rstd, scalar2=-1.0,
                                        op0=mul_, op1=mul_)
                xh = xh_pool.tile([128, C], BF16, tag="xh", name="xh")
                nc.scalar.activation(xh, xt,
                                     mybir.ActivationFunctionType.Identity,
                                     bias=nbias, scale=rstd)
                res = []
                for ci in range(CT):
                    pt = ps_tr.tile([128, 128], BF16, tag="tr", name="tr")
                    nc.tensor.transpose(pt, xh[:, ci * 128:(ci + 1) * 128], ident)
                    res.append(pt)
                return res

            # ---------------- phase 1+2: LN1 -> hT; QKV -> kT, v, qT -----
            with tc.tile_pool(name="kvq", bufs=1) as pk:
                kT = [pk.tile([128, N], BF16, tag=f"kT{c}", name=f"kT{c}")
                      for c in range(CT)]
                vv = [pk.tile([128, C], BF16, tag=f"v{t}", name=f"v{t}")
                      for t in range(NT)]
                qT = [pk.tile([128, R], BF16, tag=f"qT{c}", name=f"qT{c}")
                      for c in range(CT)]
                oT = [pk.tile([128, R], BF16, tag=f"oT{c}", name=f"oT{c}")
                      for c in range(CT)]
                qTz = [[pk.tile([128, R], BF16, tag=f"qTz{c}_{h}",
                                name=f"qTz{c}_{h}") for h in range(2)]
                       for c in range(CT)]
                for c in range(CT):
                    for h in range(2):
                        nc.vector.memset(qTz[c][h][(1 - h) * 64:(1 - h) * 64 + 64, :], 0.0)
                with tc.tile_pool(name="wqkv", bufs=1) as pwq, \
                     tc.tile_pool(name="hT", bufs=1) as ph:
                    wq_sb = [pwq.tile([128, 3 * C], BF16, tag=f"wq{c}",
                                      name=f"wq{c}") for c in range(CT)]
                    for ci in range(CT):
                        nc.sync.dma_start(
                            out=wq_sb[ci],
                            in_=P["w_qkv"][ci * 128:(ci + 1) * 128, :])
                    hT = [ph.tile([128, N], BF16, tag=f"hT{c}",
                                  name=f"hT{c}") for c in range(CT)]
                    with tc.tile_pool(name="ln1", bufs=2) as pl, \
                         tc.tile_pool(name="ln1ps", bufs=2, space="PSUM") as pltr:
                        xhs = []
                        for t in range(NT):
                            if t < QT:
                                xt = x_sb[t]
                            else:
                                xt = pl.tile([128, C], F32, tag=f"xkv{t}",
                                             name=f"xkv{t}")
                                nc.sync.dma_start(
                                    out=xt, in_=P["x"][t * 128:(t + 1) * 128, :])
                            stats = pl.tile([128, 3, 6], F32, tag="st",
                                            name="st")
                            x3 = xt.rearrange("p (s c) -> p s c", c=256)
                            for s in range(3):
                                nc.vector.bn_stats(out=stats[:, s, :],
                                                   in_=x3[:, s, :])
                            mv = pl.tile([128, 2], F32, tag="mv", name="mv")
                            nc.vector.bn_aggr(out=mv, in_=stats)
                            rstd = pl.tile([128, 1], F32, tag="rstd",
                                           name="rstd")
                            nc.scalar.activation(rstd, mv[:, 1:2], Sqrt,
                                                 bias=eps)
                            nc.vector.reciprocal(rstd, rstd)
                            nbias = pl.tile([128, 1], F32, tag="nb", name="nb")
                            nc.vector.tensor_scalar(out=nbias, in0=mv[:, 0:1],
                                                    scalar1=rstd, scalar2=-1.0,
                                                    op0=mul_, op1=mul_)
                            xh = pl.tile([128, C], BF16, tag=f"xh{t}",
                                         name=f"xh{t}")
                            nc.scalar.activation(
                                xh, xt, mybir.ActivationFunctionType.Identity,
                                bias=nbias, scale=rstd)
                            xhs.append(xh)
                        for ci in range(CT):
                            for t in range(NT):
                                pt = pltr.tile([128, 128], BF16, tag="tr",
                                               name="tr")
                                nc.tensor.transpose(
                                    pt, xhs[t][:, ci * 128:(ci + 1) * 128],
                                    ident)
                                nc.scalar.copy(
                                    hT[ci][:, t * 128:(t + 1) * 128], pt)

                    with tc.tile_pool(name="qkps", bufs=2, space="PSUM") as pq:
                        # v first: out [tok 128, vcol] (attention o-mms need it)
                        for t in range(NT):
                            for half in range(2):
                                ps = pq.tile([128, 384], F32, tag="pv",
                                             name="pv", bufs=3)
                                nc.tensor.matmul(
                                    ps, ones[0:1, 0:128],
                                    b_qkv[0:1, 2 * C + half * 384:
                                          2 * C + (half + 1) * 384],
                                    start=True, stop=False)
                                for ci in range(CT):
                                    nc.tensor.matmul(
                                        ps, hT[ci][:, t * 128:(t + 1) * 128],
                                        wq_sb[ci][:, 2 * C + half * 384:
                                                  2 * C + (half + 1) * 384],
                                        start=False, stop=(ci == CT - 1))
                                nc.vector.tensor_copy(
                                    vv[t][:, half * 384:(half + 1) * 384], ps)
                        # per ct: kT halves + kTz variants, then qT + qTz
                        for kc in range(CT):
                            for half in range(2):
                                ps = pq.tile([128, 512], F32, tag="pk",
                                             name="pk", bufs=3)
                                nc.tensor.matmul(
                                    ps,
                                    b_qkv[0:1, C + kc * 128:C + (kc + 1) * 128],
                                    ones, start=True, stop=False)
                                for ci in range(CT):
                                    nc.tensor.matmul(
                                        ps,
                                        wq_sb[ci][:, C + kc * 128:C + (kc + 1) * 128],
                                        hT[ci][:, half * 512:(half + 1) * 512],
                                        start=False, stop=(ci == CT - 1))
                                nc.vector.tensor_copy(
                                    kT[kc][:, half * 512:(half + 1) * 512], ps)
                            ps = pq.tile([128, 512], F32, tag="pq", name="pq")
                            nc.tensor.matmul(
                                ps, b_qkv[0:1, kc * 128:(kc + 1) * 128],
                                ones, start=True, stop=False)
                            for ci in range(CT):
                                nc.tensor.matmul(
                                    ps, wq_sb[ci][:, kc * 128:(kc + 1) * 128],
                                    hT[ci][:, 0:512],
                                    start=False, stop=(ci == CT - 1))
                            nc.scalar.activation(
                                qT[kc], ps, mybir.ActivationFunctionType.Copy,
                                scale=float(SCALE))
                            for hi in range(2):
                                nc.scalar.activation(
                                    qTz[kc][hi][hi * 64:hi * 64 + 64, :],
                                    ps[hi * 64:hi * 64 + 64, :],
                                    mybir.ActivationFunctionType.Copy,
                                    scale=float(SCALE))

                # proj/fc weight DMAs are interleaved between attention
                # head-pairs below so they don't block the wout queue
                with tc.tile_pool(name="wlate", bufs=1) as pwl:
                    wp_sb = [pwl.tile([128, C], BF16, tag=f"wp{c}",
                                      name=f"wp{c}") for c in range(CT)]
                    wf1_sb = [pwl.tile([128, HID], BF16, tag=f"wf1_{c}",
                                       name=f"wf1_{c}") for c in range(CT)]
                    wf2_sb = [pwl.tile([128, C], BF16, tag=f"wf2_{t}",
                                       name=f"wf2_{t}") for t in range(HT)]
                    wjobs = (
                        [(wp_sb[c], P["w_proj"][c * 128:(c + 1) * 128, :])
                         for c in range(CT)]
                        + [(wf1_sb[c], P["w_fc1"][c * 128:(c + 1) * 128, :])
                           for c in range(CT)]
                        + [(wf2_sb[t], P["w_fc2"][t * 128:(t + 1) * 128, :])
                           for t in range(HT)]
                    )

                    # ---------------- phase 3: attention (head pairs) -----
                    with tc.tile_pool(name="att", bufs=4) as pa, \
                         tc.tile_pool(name="attE", bufs=4) as pe, \
                         tc.tile_pool(name="aps", bufs=1, space="PSUM") as pps, \
                         tc.tile_pool(name="apo", bufs=1, space="PSUM") as ppo:
                        nw = (len(wjobs) + H // 2 - 2) // (H // 2 - 1)
                        for hp in range(H // 2):
                            ct = hp
                            heads = (2 * hp, 2 * hp + 1)
                            if hp >= 1:
                                for dst, src in wjobs[(hp - 1) * nw:hp * nw]:
                                    nc.sync.dma_start(out=dst, in_=src)
                            # ---- q path: scores -> exp(+Z) -> norm -> DMA
                            zs = {}
                            for qt in range(QT):
                                pq2 = {}
                                for hi in range(2):
                                    pq2[hi] = pps.tile([128, N], F32,
                                                       tag=f"sq{hi}",
                                                       name=f"sq{hi}", bufs=1)
                                for half in range(2):
                                    for hi in range(2):
                                        nc.tensor.matmul(
                                            pq2[hi][:, half * 512:(half + 1) * 512],
                                            qTz[ct][hi][:, qt * 128:(qt + 1) * 128],
                                            kT[ct][:, half * 512:(half + 1) * 512],
                                            start=True, stop=True)
                                for hi in range(2):
                                    if qt == 0:
                                        nc.vector.tensor_mul(
                                            pq2[hi][0:3, 0:3], pq2[hi][0:3, 0:3],
                                            mask_mul)
                                    e = pa.tile([128, N], F32, tag="e", name="e")
                                    z = pa.tile([128, 1], F32, tag="z",
                                                name="z", bufs=10)
                                    nc.scalar.activation(e, pq2[hi], Exp,
                                                         accum_out=z)
                                    nc.vector.reciprocal(z, z)
                                    zs[(qt, hi)] = z
                                    nc.vector.tensor_scalar_mul(e, in0=e,
                                                                scalar1=z)
                                    nc.sync.dma_start(
                                        out=wout[heads[hi],
                                                 qt * 128:(qt + 1) * 128, :],
                                        in_=e)
                            # ---- 1/Z broadcast rows [64, 512] per head
                            zbc = {}
                            for hi in range(2):
                                zrow = pa.tile([1, 512], BF16, tag=f"zr{hi}",
                                               name=f"zr{hi}", bufs=2)
                                for qt in range(QT):
                                    pzt = ppo.tile([1, 128], F32,
                                                   tag="zoo",
                                                   name=f"zt{hi}", bufs=2)
                                    nc.tensor.matmul(pzt, zs[(qt, hi)], ident_f,
                                                     is_transpose=True,
                                                     start=True, stop=True)
                                    nc.vector.tensor_copy(
                                        zrow[0:1, qt * 128:(qt + 1) * 128], pzt)
                                pbc = ppo.tile([64, 512], F32, tag="zoo",
                                               name=f"bc{hi}", bufs=2)
                                nc.tensor.matmul(pbc, ones[0:1, 0:64], zrow,
                                                 start=True, stop=True)
                                zb = pa.tile([64, 512], F32, tag=f"zbc{hi}",
                                             name=f"zbc{hi}", bufs=2)
                                nc.vector.tensor_copy(zb, pbc)
                                zbc[hi] = zb
                            # ---- T path + o accumulation, per k-tile
                            so_pair = ppo.tile([128, 512], F32, tag="zoo",
                                               name="so_pair", bufs=2)
                            so = {0: so_pair[0:64, :], 1: so_pair[64:128, :]}
                            for kt in range(KT):
                                pss = []
                                for hi in range(2):
                                    ps = pps.tile([128, 512], F32,
                                                  tag=f"sT{hi}", name=f"sT{hi}",
                                                  bufs=1)
                                    nc.tensor.matmul(
                                        ps,
                                        kT[ct][:, kt * 128:(kt + 1) * 128],
                                        qTz[ct][hi],
                                        start=True, stop=True)
                                    pss.append(ps)
                                ets = []
                                for hi in range(2):
                                    e_t = pe.tile([128, 512], BF16, tag="eT",
                                                  name="eT")
                                    nc.scalar.activation(e_t, pss[hi], Exp)
                                    if kt == 0:
                                        nc.vector.tensor_mul(
                                            e_t[0:3, 0:3], e_t[0:3, 0:3],
                                            mask_mul)
                                        nc.vector.tensor_add(
                                            e_t[0:3, 0:3], e_t[0:3, 0:3],
                                            mask_add)
                                    ets.append(e_t)
                                for hi in range(2):
                                    nc.tensor.matmul(
                                        so[hi],
                                        vv[kt][:, heads[hi] * 64:
                                               heads[hi] * 64 + 64],
                                        ets[hi],
                                        start=(kt == 0), stop=(kt == KT - 1),
                                        tile_position=(0, hi * 64))
                            # ---- normalize into oT
                            for hi in range(2):
                                nc.vector.tensor_mul(
                                    oT[ct][hi * 64:hi * 64 + 64, :],
                                    so[hi], zbc[hi])

                    # ------------- phase 4: proj + residual + LN2 ---------
                    with tc.tile_pool(name="prl", bufs=2) as pjl, \
                         tc.tile_pool(name="pjps", bufs=2, space="PSUM") as pjp:
                        for qt in range(QT):
                            for half, w0, w1 in ((0, 0, 512), (1, 512, 768)):
                                ps = pjp.tile([128, w1 - w0], F32,
                                              tag=f"pp{half}", name=f"pp{half}")
                                nc.tensor.matmul(ps, ones[0:1, 0:128],
                                                 b_proj[0:1, w0:w1],
                                                 start=True, stop=False)
                                for ci in range(CT):
                                    nc.tensor.matmul(
                                        ps, oT[ci][:, qt * 128:(qt + 1) * 128],
                                        wp_sb[ci][:, w0:w1],
                                        start=False, stop=(ci == CT - 1))
                                nc.vector.tensor_add(
                                    x2[qt][:, w0:w1], ps, x_sb[qt][:, w0:w1])
                            pts = layernorm_to(x2[qt], pjl, pjp)
                            for ci in range(CT):
                                nc.vector.tensor_copy(
                                    h2T[ci][:, qt * 128:(qt + 1) * 128],
                                    pts[ci])

                    # ---------------- phase 5: MLP ------------------------
                    with tc.tile_pool(name="mlg", bufs=1) as pg, \
                         tc.tile_pool(name="mps", bufs=2, space="PSUM") as pmp, \
                         tc.tile_pool(name="mpsa", bufs=1, space="PSUM") as pma:
                        gT = [pg.tile([128, R], BF16, tag=f"gT{t}",
                                      name=f"gT{t}") for t in range(HT)]
                        acc = [pma.tile([128, 384], F32, tag=f"acc{q}",
                                        name=f"acc{q}") for q in range(QT)]
                        for pass_i, w0, w1 in ((0, 0, 384), (1, 384, 768)):
                            for qt in range(QT):
                                nc.tensor.matmul(acc[qt], ones[0:1, 0:128],
                                                 b_fc2[0:1, w0:w1],
                                                 start=True, stop=False)
                            for ht in range(HT):
                                if pass_i == 0:
                                    ps = pmp.tile([128, 512], F32, tag="f1",
                                                  name="f1")
                                    nc.tensor.matmul(
                                        ps, b_fc1[0:1, ht * 128:(ht + 1) * 128],
                                        ones, start=True, stop=False)
                                    for ci in range(CT):
                                        nc.tensor.matmul(
                                            ps,
                                            wf1_sb[ci][:, ht * 128:(ht + 1) * 128],
                                            h2T[ci],
                                            start=False, stop=(ci == CT - 1))
                                    nc.scalar.activation(gT[ht], ps, Gelu)
                                for qt in range(QT):
                                    nc.tensor.matmul(
                                        acc[qt],
                                        gT[ht][:, qt * 128:(qt + 1) * 128],
                                        wf2_sb[ht][:, w0:w1],
                                        start=False, stop=(ht == HT - 1))
                            for qt in range(QT):
                                nc.vector.tensor_add(x2[qt][:, w0:w1],
                                                     acc[qt],
                                                     x2[qt][:, w0:w1])
                                nc.sync.dma_start(
                                    out=xout[qt * 128:(qt + 1) * 128, w0:w1],
                                    in_=x2[qt][:, w0:w1])

    _fixup_excess_waits(nc)
    return nc


_NC = None


def _get_nc():
    global _NC
    if _NC is None:
        _NC = _build()
    return _NC


def _prep_in_maps(inputs):
    import ml_dtypes
    bf = ml_dtypes.bfloat16
    x = np.ascontiguousarray(np.asarray(inputs["x"], dtype=np.float32))
    ln1_g = np.asarray(inputs["ln1_g"], np.float32)
    ln1_b = np.asarray(inputs["ln1_b"], np.float32)
    ln2_g = np.asarray(inputs["ln2_g"], np.float32)
    ln2_b = np.asarray(inputs["ln2_b"], np.float32)
    w_qkv = np.asarray(inputs["w_qkv"], np.float32)
    w_proj = np.asarray(inputs["w_proj"], np.float32)
    b_proj = np.asarray(inputs["b_proj"], np.float32)
    w_fc1 = np.asarray(inputs["w_fc1"], np.float32)
    b_fc1 = np.asarray(inputs["b_fc1"], np.float32)
    w_fc2 = np.asarray(inputs["w_fc2"], np.float32)
    b_fc2 = np.asarray(inputs["b_fc2"], np.float32)

    # fold LN affine into following matmuls
    w_qkv_f = w_qkv * ln1_g[:, None]
    b_qkv_f = (ln1_b @ w_qkv)[None, :]
    w_fc1_f = w_fc1 * ln2_g[:, None]
    b_fc1_f = (b_fc1 + ln2_b @ w_fc1)[None, :]

    cc = np.ascontiguousarray
    eye3 = np.eye(3, dtype=np.float32)
    common = dict(
        w_qkv=cc(w_qkv_f.astype(bf)), w_proj=cc(w_proj.astype(bf)),
        w_fc1=cc(w_fc1_f.astype(bf)), w_fc2=cc(w_fc2.astype(bf)),
        b_qkv=cc(b_qkv_f.astype(bf)), b_proj=cc(b_proj[None, :].astype(bf)),
        b_fc1=cc(b_fc1_f.astype(bf)), b_fc2=cc(b_fc2[None, :].astype(bf)),
        ident=np.eye(128, dtype=np.float32).astype(bf),
        ident_f=np.eye(128, dtype=np.float32),
        ones=np.ones((1, 512), np.float32).astype(bf),
    )
    in_maps = []
    for core in range(8):
        b, j = core // 2, core % 2
        xb = x[b]
        if j == 1:
            xb = np.ascontiguousarray(np.concatenate([xb[R:], xb[:R]], axis=0))
        m = dict(common)
        m["x"] = np.ascontiguousarray(xb)
        if j == 0:
            m["mask_mul"] = cc(eye3.astype(bf))
            m["mask_add"] = cc((1.0 - eye3).astype(bf))
        else:
            m["mask_mul"] = cc(np.ones((3, 3), np.float32).astype(bf))
            m["mask_add"] = cc(np.zeros((3, 3), np.float32).astype(bf))
        in_maps.append(m)
    return in_maps


def _assemble(results):
    x_out = np.empty((B, N, C), np.float32)
    weights = np.empty((B, H, N, N), np.float32)
    for core in range(8):
        b, j = core // 2, core % 2
        r = results[core]
        x_out[b, j * R:(j + 1) * R] = r["xout"]
        w = r["wout"]
        if j == 1:
            w = np.concatenate([w[:, :, R:], w[:, :, :R]], axis=2)
        weights[b, :, j * R:(j + 1) * R, :] = w
    return x_out, weights


def run(inputs, trace=False, tmpdir=None):
    nc = _get_nc()
    in_maps = _prep_in_maps(inputs)
    res = run_bass_kernel_spmd(nc, in_maps, core_ids=list(range(8)),
                               trace=trace, tmpdir=tmpdir)
    return _assemble(res.results), res


def kernel(**inputs):
    (x_out, weights), _ = run(inputs)
    return x_out, weights
